# revision 53
# baseline (speedup 1.0000x reference)
"""Multi-head dot-product attention (B=2, S=2048, D=2048, H=16, HD=128) with
RoPE + causal mask, sharded over 8 NeuronCores: batch (2) x head-groups (4).

Each core computes 4 heads of one batch element end-to-end (QKV projections,
RoPE, causal softmax attention, output projection); the host sums the four
head-group partials per batch element.

Schedule notes:
- The attention stream is exp(Scalar)-paced: per key-tile the PE does ~1.7us
  of matmul while Scalar does ~2.4us of exp.  Since the PE executes its
  queue in order, dense work must be interleaved INTO the attention stream
  to fill those bubbles: the Q-projection of the next block and the output
  projection of the previous block are emitted as small units between
  key-tile iterations.
- Softmax row-sum + partition-broadcast is one matmul with an all-ones
  [128,128] lhsT (every output partition = column sum).
- Startup: only wk + the first xkvT quarter are on the critical path; other
  loads ride queues that stay out of their way.  The first K-projection
  quarter runs chunk-major across heads so the PE starts on the first
  chunk-group DMA instead of waiting for the whole quarter.

Self-contained: hardcodes all shapes; builds/compiles the Bass program once
per process and runs it via run_bass_kernel_spmd on cores 0-7.
"""

import os
import sys
import types

import ml_dtypes
import numpy as np

B, S, D, H, HD = 2, 2048, 2048, 16, 128
HPC = 4                 # heads per core
HW = HPC * HD           # 512: per-core projection width
NQB = S // 512          # 4 query blocks / token quarters of 512
NKT = S // 128          # 16 key-token tiles of 128
NDC = D // 128          # 16 contraction chunks of 128
N_CORES = 8
SCALE = float(HD) ** -0.5

BF16 = ml_dtypes.bfloat16

_CACHE = {}


def _install_ntff_hook():
    """The image's antenv lacks axon_hooks, so boot() couldn't register the
    NTFF profile hook; recreate the module + hook so trace=True works."""
    if "antenv.axon_hooks" in sys.modules:
        return
    try:
        import antenv  # noqa: F401
        mod = types.ModuleType("antenv.axon_hooks")
        _h = [None]
        mod.set_axon_ntff_profile_hook = lambda h: _h.__setitem__(0, h)
        mod.get_axon_ntff_profile_hook = lambda: _h[0]
        sys.modules["antenv.axon_hooks"] = mod
        from trn_agent_boot.trn_boot import _ntff_profile_via_ctypes
        mod.set_axon_ntff_profile_hook(
            _ntff_profile_via_ctypes("/opt/axon/libaxon_pjrt.so"))
    except Exception:
        pass


def _build():
    import concourse.mybir as mybir
    import concourse.tile as tile
    from concourse import bacc

    f32 = mybir.dt.float32
    bf16 = mybir.dt.bfloat16
    fp16 = mybir.dt.float16
    Exp = mybir.ActivationFunctionType.Exp

    nc = bacc.Bacc("TRN2", target_bir_lowering=False, debug=False,
                   enable_asserts=True, num_devices=N_CORES)

    dram = {}
    for name, shape, dt in [
        ("xqT", [D, S], bf16), ("xkvT", [D, S], bf16),
        ("wq", [D, HW], bf16), ("wk", [D, HW], bf16), ("wv", [D, HW], bf16),
        ("wo", [HW, D], bf16),
        ("trigT", [HD, 2 * S], bf16),  # sin | cos, one DMA
        ("rmatT", [HD, HD], bf16),
        ("ones_sq", [128, 128], fp16),
        ("maskt", [128, 128], fp16),
    ]:
        dram[name] = nc.dram_tensor(name, shape, dt, kind="ExternalInput").ap()
    outp = nc.dram_tensor("outp", [S, D], bf16, kind="ExternalOutput").ap()

    with tile.TileContext(nc) as tc:
        with (
            tc.tile_pool(name="const", bufs=1) as cpool,
            tc.tile_pool(name="kt", bufs=1) as kt_pool,
            tc.tile_pool(name="qt", bufs=1) as qt_pool,
            tc.tile_pool(name="vsb", bufs=1) as v_pool,
            tc.tile_pool(name="ctxn", bufs=1) as ctx_pool,
            tc.tile_pool(name="wkv", bufs=1) as wkv_pool,
            tc.tile_pool(name="xin", bufs=2) as xpool,
            tc.tile_pool(name="raw", bufs=8) as raw_pool,
            tc.tile_pool(name="t12", bufs=8) as t12_pool,
            tc.tile_pool(name="pp", bufs=8) as ppool,
            tc.tile_pool(name="sacc", bufs=6) as sacc_pool,
            tc.tile_pool(name="rcp", bufs=3) as rpool,
            tc.tile_pool(name="osb", bufs=6) as opool,
            # one PSUM pool for the whole kernel: 4 tags x 2 bufs = 8 banks
            tc.tile_pool(name="ps", space="PSUM", bufs=2) as pspool,
        ):
            def load_chunks(pool, name, nch, width, tag=None, eng=None,
                            groups=None):
                t = pool.tile([128, nch * width], bf16, tag=tag or name,
                              name=name + "_sb")
                dv = dram[name].rearrange("(n p) w -> p n w", p=128)
                if groups is None:
                    step = min(8, nch)
                    groups = [(i, step) for i in range(0, nch, step)]
                for i, step in groups:
                    e = eng or nc.sync
                    e.dma_start(t[:, i * width:(i + step) * width],
                                dv[:, i:i + step, :])
                return t

            def load(name, shape, dt=bf16, eng=None):
                t = cpool.tile(shape, dt, tag=name, name=name)
                (eng or nc.gpsimd).dma_start(t[:], dram[name][:])
                return t

            # Per-queue DMAs serialize (issue ~1.5us + transfer), so use
            # few, large transfers ordered by first consumption:
            #  scalar: wk (fine head), rmatT, trigT
            #  sync:   xkvT q0 chunks 0-7 (fine head), then x q1..q3
            #  gpsimd: xkvT q0 chunks 8-15, wv, then the WAR-gated wq and
            #          behind it wo/maskt/ones (transfer in the phase-1
            #          DMA lull, far ahead of their phase-2 consumers)
            wk_sb = load_chunks(wkv_pool, "wk", NDC, HW, eng=nc.scalar,
                                groups=[(0, 2), (2, 2), (4, 4), (8, 8)])
            rmatT = load("rmatT", [HD, HD], eng=nc.scalar)
            wv_sb = load_chunks(wkv_pool, "wv", NDC, HW, eng=nc.scalar,
                                groups=[(i, 4) for i in range(0, NDC, 4)])
            trigT = load("trigT", [HD, 2 * S], eng=nc.scalar)
            maskt = load("maskt", [128, 128], fp16, eng=nc.scalar)
            ones_sq = load("ones_sq", [128, 128], fp16, eng=nc.scalar)
            wo_sb = load_chunks(cpool, "wo", HW // 128, D, eng=nc.scalar,
                                groups=[(0, 2), (2, 2)])
            # wq reuses wk's buffer; its DMA WAR-waits on the last wk read
            # (end of the K projections) on the gpsimd queue, where the
            # stall blocks nothing (phase-2 accs ops start much later).
            wq_sb = load_chunks(wkv_pool, "wq", NDC, HW, tag="wk",
                                eng=nc.gpsimd,
                                groups=[(i, 4) for i in range(0, NDC, 4)])

            # per-head projection outputs (+rope for Q/K)
            kt_sb = [kt_pool.tile([128, S], bf16, tag=f"kt{h}", name=f"kt{h}")
                     for h in range(HPC)]
            qt_sb = [qt_pool.tile([128, S], bf16, tag=f"qt{h}", name=f"qt{h}")
                     for h in range(HPC)]
            v_sb = v_pool.tile([128, NKT * HW], fp16, tag="v", name="v_sb")
            ctx_sb = [ctx_pool.tile([128, S], bf16, tag=f"ctx{h}",
                                    name=f"ctx{h}") for h in range(HPC)]

            def rope(raw, out_sl, tq, rot_tag):
                ssl = slice(tq * 512, (tq + 1) * 512)
                csl = slice(S + tq * 512, S + (tq + 1) * 512)
                rot = pspool.tile([128, 512], f32, tag=rot_tag, name="rot")
                nc.tensor.matmul(rot[:], lhsT=rmatT[:], rhs=raw[:])
                t1 = t12_pool.tile([128, 512], bf16, tag="t1", name="t1")
                nc.vector.tensor_mul(t1[:], rot[:], trigT[:, ssl])
                t2 = t12_pool.tile([128, 512], bf16, tag="t2", name="t2")
                nc.vector.tensor_mul(t2[:], raw[:], trigT[:, csl])
                nc.vector.tensor_add(out_sl, t1[:], t2[:])

            def load_x(xname, tq):
                xt = xpool.tile([128, NDC * 512], bf16, tag="xin",
                                name=f"{xname}_{tq}")
                xv = dram[xname].rearrange("(n p) s -> p n s", p=128)
                for kc in range(0, NDC, 4):
                    nc.sync.dma_start(
                        xt[:, kc * 512:(kc + 4) * 512],
                        xv[:, kc:kc + 4, tq * 512:(tq + 1) * 512])
                return xt

            def mm_proj(ps, w_sb, xt, h, kc):
                nc.tensor.matmul(
                    ps[:],
                    lhsT=w_sb[:, kc * HW + h * HD:kc * HW + (h + 1) * HD],
                    rhs=xt[:, kc * 512:(kc + 1) * 512],
                    start=(kc == 0), stop=(kc == NDC - 1))

            def proj_units(xname, tq, xt, w_sb, out_tiles, acc_tag, rot_tag,
                           raw_eng=None):
                """Generator: 512-wide per-head projection (+rope) of token
                quarter tq, yielding after ~half-chain units.  The x DMA is
                issued by the caller (load_x) so the transfer overlaps the
                preceding block.  The PSUM->SBUF raw copy of chain h is
                emitted a full chain after h completes, so when these units
                are interleaved into the attention stream the copy's engine
                (Vector in phase 2, so Scalar stays exp-only) never waits on
                an unfinished chain and stalls unrelated attention ops
                queued behind it."""
                def finish(ps, h):
                    raw = raw_pool.tile([128, 512], bf16, tag="raw",
                                        name=f"raw_{xname}_{tq}_{h}")
                    if raw_eng == "vector":
                        nc.vector.tensor_copy(raw[:], ps[:])
                    else:
                        nc.scalar.copy(raw[:], ps[:])
                    rope(raw, out_tiles[h][:, tq * 512:(tq + 1) * 512],
                         tq, rot_tag)

                prev = None
                for h in range(HPC):
                    ps = pspool.tile([128, 512], f32, tag=acc_tag,
                                     name=f"ps_{xname}_{tq}_{h}")
                    for kc in range(8):
                        mm_proj(ps, w_sb, xt, h, kc)
                    yield
                    for kc in range(8, NDC):
                        mm_proj(ps, w_sb, xt, h, kc)
                    yield
                    if prev is not None:
                        finish(*prev)
                        yield
                    prev = (ps, h)
                finish(*prev)

            def proj_cold(tq, xt):
                """Chunk-major K projection + V for the startup quarter:
                consumes each chunk-group DMA as it lands.  K psums on
                A,A,B,B; rope rot on C; V chunk-major on C,C,D,D."""
                pss = [pspool.tile([128, 512], f32,
                                   tag=("A" if h < 2 else "B"),
                                   name=f"psc_{h}") for h in range(HPC)]
                for g in range(0, NDC, 4):
                    for h in range(HPC):
                        for kc in range(g, g + 4):
                            mm_proj(pss[h], wk_sb, xt, h, kc)
                for h in range(HPC):
                    raw = raw_pool.tile([128, 512], bf16, tag="raw",
                                        name=f"rawc_{h}")
                    nc.scalar.copy(raw[:], pss[h][:])
                    rope(raw, kt_sb[h][:, tq * 512:(tq + 1) * 512], tq, "C")
                # V chunk-major
                vps = [pspool.tile([128, 512], f32,
                                   tag=("C" if ti < 2 else "D"),
                                   name=f"vpsc_{ti}") for ti in range(4)]
                for g in range(0, NDC, 4):
                    for ti in range(4):
                        for kc in range(g, g + 4):
                            nc.tensor.matmul(
                                vps[ti][:],
                                lhsT=xt[:, kc * 512 + ti * 128:
                                        kc * 512 + (ti + 1) * 128],
                                rhs=wv_sb[:, kc * HW:(kc + 1) * HW],
                                start=(kc == 0), stop=(kc == NDC - 1))
                for ti in range(4):
                    t = tq * 4 + ti
                    nc.scalar.copy(v_sb[:, t * HW:(t + 1) * HW], vps[ti][:])

            def emit_v(tq, xt):
                for ti in range(4):
                    t = tq * 4 + ti
                    ps = pspool.tile([128, 512], f32, tag="C",
                                     name=f"vps_{t}")
                    for kc in range(NDC):
                        nc.tensor.matmul(
                            ps[:],
                            lhsT=xt[:, kc * 512 + ti * 128:
                                    kc * 512 + (ti + 1) * 128],
                            rhs=wv_sb[:, kc * HW:(kc + 1) * HW],
                            start=(kc == 0), stop=(kc == NDC - 1))
                    nc.scalar.copy(v_sb[:, t * HW:(t + 1) * HW], ps[:])

            def attention_block(qb, fill, ration=1):
                """Causal attention for query block qb; heads in pairs
                (ctx psums A,A / st C,C).  The PV matmuls lag one key-tile
                behind the logits and one dense fill unit is pulled between
                them, so the exp latency is covered twice over.  The softmax
                numerator accumulation runs on the idle Pool engine so the
                Vector stream stays short-latency (it gates PV via the
                causal mask) and Scalar stays exp-only."""
                qsl = slice(qb * 512, (qb + 1) * 512)
                last = 4 * qb + 3
                for hp in range(2):
                    pair = (2 * hp, 2 * hp + 1)
                    ctx_ps = {h: pspool.tile([128, 512], f32, tag="A",
                                             name=f"ctxps_{h}_{qb}")
                              for h in pair}
                    accs = {h: sacc_pool.tile([128, 512], fp16, tag="acc",
                                              name=f"acc_{h}_{qb}")
                            for h in pair}

                    def pv(kt, ps):
                        off = 128 * (kt - 4 * qb) if kt >= 4 * qb else 0
                        for h in pair:
                            nc.tensor.matmul(
                                ctx_ps[h][:, off:],
                                lhsT=v_sb[:, kt * HW + h * HD:
                                          kt * HW + (h + 1) * HD],
                                rhs=ps[h][:, off:], start=(kt == 0),
                                stop=(kt == last))

                    prev = None
                    for kt in range(last + 1):
                        off = 128 * (kt - 4 * qb) if kt >= 4 * qb else 0
                        cur = {}
                        for h in pair:
                            st = pspool.tile([128, 512], f32, tag="C",
                                             name=f"st_{h}_{qb}_{kt}")
                            nc.tensor.matmul(
                                st[:, off:],
                                lhsT=kt_sb[h][:, kt * 128:(kt + 1) * 128],
                                rhs=qt_sb[h][:, qb * 512 + off:
                                             (qb + 1) * 512])
                            p = ppool.tile([128, 512], fp16, tag="p",
                                           name=f"p_{h}_{qb}_{kt}")
                            nc.scalar.activation(p[:, off:], st[:, off:],
                                                 Exp, scale=SCALE)
                            if kt >= 4 * qb:
                                nc.vector.tensor_mul(
                                    p[:, off:off + 128],
                                    p[:, off:off + 128], maskt[:])
                            # numerator row-sum accumulation split across
                            # the Pool and Vector engines (Pool alone is
                            # ~2x slower per op and becomes the pacer)
                            acc_eng = nc.gpsimd if h % 2 == 0 else nc.vector
                            if kt == 0:
                                acc_eng.tensor_copy(accs[h][:], p[:])
                            else:
                                acc_eng.tensor_add(accs[h][:, off:],
                                                   accs[h][:, off:],
                                                   p[:, off:])
                            cur[h] = p
                        if (kt + hp) % ration == 0:
                            next(fill, None)
                        if prev is not None:
                            pv(*prev)
                        prev = (kt, cur)
                    pv(*prev)
                    next(fill, None)
                    for h in pair:
                        # one matmul: every partition of s_bc = row-sum of
                        # the softmax numerators (all-ones lhsT broadcasts)
                        s_bc = pspool.tile([128, 512], f32, tag="D",
                                           name=f"sbc_{h}_{qb}")
                        nc.tensor.matmul(s_bc[:], lhsT=ones_sq[:],
                                         rhs=accs[h][:])
                        rb_sb = rpool.tile([128, 512], f32, tag="rb",
                                           name=f"rbsb_{h}_{qb}")
                        nc.vector.reciprocal_approx_fast(rb_sb[:], s_bc[:])
                        nc.vector.tensor_mul(ctx_sb[h][:, qsl],
                                             ctx_ps[h][:], rb_sb[:])
                        next(fill, None)

            def wo_units(qb):
                for qt in range(qb * 4, qb * 4 + 4):
                    for db in range(NQB):
                        ps = pspool.tile([128, 512], f32, tag="D",
                                         name=f"ops_{qt}_{db}")
                        for h in range(HPC):
                            nc.tensor.matmul(
                                ps[:],
                                lhsT=ctx_sb[h][:, qt * 128:(qt + 1) * 128],
                                rhs=wo_sb[:, h * D + db * 512:
                                          h * D + (db + 1) * 512],
                                start=(h == 0), stop=(h == HPC - 1))
                        osb = opool.tile([128, 512], bf16, tag="o",
                                         name=f"osb_{qt}_{db}")
                        # scalar copy: by unit construction the wo psum is
                        # already complete, so this never blocks the exps
                        # queued behind it
                        # alternate copy engines: each insertion delays the
                        # host stream (exps on Scalar, masks on Vector) by
                        # ~0.6us, so split the load between them
                        if (qt + db) % 2:
                            nc.scalar.copy(osb[:], ps[:])
                        else:
                            nc.vector.tensor_copy(osb[:], ps[:])
                        nc.sync.dma_start(
                            outp[qt * 128:(qt + 1) * 128,
                                 db * 512:(db + 1) * 512], osb[:])
                        yield

            def drain(g):
                for _ in g:
                    pass

            def chain(*gens):
                for g in gens:
                    yield from g

            # ---- phase 1: K^T + V (stream xkvT) ----
            proj_cold(0, load_x("xkvT", 0))
            for tq in range(1, NQB):
                xt = load_x("xkvT", tq)
                drain(proj_units("xkvT", tq, xt, wk_sb, kt_sb, "A", "D"))
                emit_v(tq, xt)

            # ---- phase 2: attention blocks q3..q0, with the next block's
            # Q-projection and the previous block's output projection
            # interleaved into each block's exp-paced stream ----
            xq3 = load_x("xqT", 3)
            drain(proj_units("xqT", 3, xq3, wq_sb, qt_sb, "B", "D"))
            xq2 = load_x("xqT", 2)
            fill = proj_units("xqT", 2, xq2, wq_sb, qt_sb, "B", "D",
                              raw_eng="vector")
            # only 12 fill units for 36 pull sites: ration so they stretch
            # to the block-end row-sum waits
            attention_block(3, fill, ration=3)
            drain(fill)
            xq1 = load_x("xqT", 1)
            fill = chain(wo_units(3),
                         proj_units("xqT", 1, xq1, wq_sb, qt_sb, "B", "D",
                                    raw_eng="vector"))
            attention_block(2, fill)
            drain(fill)
            xq0 = load_x("xqT", 0)
            fill = chain(wo_units(2),
                         proj_units("xqT", 0, xq0, wq_sb, qt_sb, "B", "D",
                                    raw_eng="vector"))
            attention_block(1, fill)
            drain(fill)
            fill = wo_units(1)
            attention_block(0, fill)
            drain(fill)
            drain(wo_units(0))

    nc.compile()
    return nc


def _host_constants():
    # sin/cos tables exactly as the flaxformer reference (fp32 math)
    fraction = np.arange(0, HD, 2, dtype=np.float32) / np.float32(HD)
    timescale = (np.float32(10000.0) ** fraction).astype(np.float32)
    sinusoid = np.einsum(
        "i,j->ij", np.arange(S, dtype=np.float32),
        (np.float32(1.0) / timescale)).astype(np.float32)
    sinusoid = np.concatenate([sinusoid, sinusoid], axis=-1)  # [S, HD]
    trigT = np.concatenate([np.sin(sinusoid), np.cos(sinusoid)],
                           axis=0).astype(np.float32).T.copy()  # [HD, 2S]

    # rotate_half as a matmul: rot = R @ x, lhsT = R^T
    R = np.zeros((HD, HD), np.float32)
    for i in range(64):
        R[i, i + 64] = -1.0
        R[i + 64, i] = 1.0

    # causal triangle for the 128-wide diagonal sub-block: allowed iff c >= r
    r = np.arange(128)[:, None]
    c = np.arange(128)[None, :]
    maskt = (c - r >= 0).astype(np.float32)

    return {
        "trigT": trigT.astype(BF16),
        "rmatT": R.T.copy().astype(BF16),
        "ones_sq": np.ones((128, 128), np.float16),
        "maskt": maskt.astype(np.float16),
    }


def kernel(inputs_q, inputs_kv, wq, wk, wv, wo, mask=None):
    _install_ntff_hook()
    from concourse import bass_utils

    if "nc" not in _CACHE:
        _CACHE["nc"] = _build()
        _CACHE["consts"] = _host_constants()
    nc = _CACHE["nc"]
    consts = _CACHE["consts"]

    wq2 = np.asarray(wq, np.float32).reshape(D, H * HD)
    wk2 = np.asarray(wk, np.float32).reshape(D, H * HD)
    wv2 = np.asarray(wv, np.float32).reshape(D, H * HD)
    wo2 = np.asarray(wo, np.float32).reshape(H * HD, D)
    xq = np.asarray(inputs_q, np.float32)
    xkv = np.asarray(inputs_kv, np.float32)

    in_maps = []
    for c in range(N_CORES):
        b, hg = divmod(c, H // HPC)
        hs = slice(hg * HW, (hg + 1) * HW)
        in_maps.append({
            "xqT": np.ascontiguousarray(xq[b].T).astype(BF16),
            "xkvT": np.ascontiguousarray(xkv[b].T).astype(BF16),
            "wq": wq2[:, hs].astype(BF16),
            "wk": wk2[:, hs].astype(BF16),
            "wv": wv2[:, hs].astype(BF16),
            "wo": wo2[hs, :].astype(BF16),
            **consts,
        })

    trace = bool(int(os.environ.get("KERNEL_TRACE", "0")))
    res = bass_utils.run_bass_kernel_spmd(
        nc, in_maps, core_ids=list(range(N_CORES)), trace=trace)
    _CACHE["last_result"] = res

    out = np.zeros((B, S, D), np.float32)
    for c in range(N_CORES):
        out[c // (H // HPC)] += np.asarray(res.results[c]["outp"], np.float32)
    return out


# revision 55
# speedup vs baseline: 1.0024x; 1.0024x over previous
"""Multi-head dot-product attention (B=2, S=2048, D=2048, H=16, HD=128) with
RoPE + causal mask, sharded over 8 NeuronCores: batch (2) x head-groups (4).

Each core computes 4 heads of one batch element end-to-end (QKV projections,
RoPE, causal softmax attention, output projection); the host sums the four
head-group partials per batch element.

Schedule notes:
- The attention stream is exp(Scalar)-paced: per key-tile the PE does ~1.7us
  of matmul while Scalar does ~2.4us of exp.  Since the PE executes its
  queue in order, dense work must be interleaved INTO the attention stream
  to fill those bubbles: the Q-projection of the next block and the output
  projection of the previous block are emitted as small units between
  key-tile iterations.
- Softmax row-sum + partition-broadcast is one matmul with an all-ones
  [128,128] lhsT (every output partition = column sum).
- Startup: only wk + the first xkvT quarter are on the critical path; other
  loads ride queues that stay out of their way.  The first K-projection
  quarter runs chunk-major across heads so the PE starts on the first
  chunk-group DMA instead of waiting for the whole quarter.

Self-contained: hardcodes all shapes; builds/compiles the Bass program once
per process and runs it via run_bass_kernel_spmd on cores 0-7.
"""

import os
import sys
import types

import ml_dtypes
import numpy as np

B, S, D, H, HD = 2, 2048, 2048, 16, 128
HPC = 4                 # heads per core
HW = HPC * HD           # 512: per-core projection width
NQB = S // 512          # 4 query blocks / token quarters of 512
NKT = S // 128          # 16 key-token tiles of 128
NDC = D // 128          # 16 contraction chunks of 128
N_CORES = 8
SCALE = float(HD) ** -0.5

BF16 = ml_dtypes.bfloat16

_CACHE = {}


def _install_ntff_hook():
    """The image's antenv lacks axon_hooks, so boot() couldn't register the
    NTFF profile hook; recreate the module + hook so trace=True works."""
    if "antenv.axon_hooks" in sys.modules:
        return
    try:
        import antenv  # noqa: F401
        mod = types.ModuleType("antenv.axon_hooks")
        _h = [None]
        mod.set_axon_ntff_profile_hook = lambda h: _h.__setitem__(0, h)
        mod.get_axon_ntff_profile_hook = lambda: _h[0]
        sys.modules["antenv.axon_hooks"] = mod
        from trn_agent_boot.trn_boot import _ntff_profile_via_ctypes
        mod.set_axon_ntff_profile_hook(
            _ntff_profile_via_ctypes("/opt/axon/libaxon_pjrt.so"))
    except Exception:
        pass


def _build():
    import concourse.mybir as mybir
    import concourse.tile as tile
    from concourse import bacc

    f32 = mybir.dt.float32
    bf16 = mybir.dt.bfloat16
    fp16 = mybir.dt.float16
    Exp = mybir.ActivationFunctionType.Exp

    nc = bacc.Bacc("TRN2", target_bir_lowering=False, debug=False,
                   enable_asserts=True, num_devices=N_CORES)

    dram = {}
    for name, shape, dt in [
        ("xqT", [D, S], bf16), ("xkvT", [D, S], bf16),
        ("wq", [D, HW], bf16), ("wk", [D, HW], bf16), ("wv", [D, HW], bf16),
        ("wo", [HW, D], bf16),
        ("trigT", [HD, 2 * S], bf16),  # sin | cos, one DMA
        ("rmatT", [HD, HD], bf16),
        ("ones_sq", [128, 128], fp16),
        ("maskt", [128, 128], fp16),
    ]:
        dram[name] = nc.dram_tensor(name, shape, dt, kind="ExternalInput").ap()
    outp = nc.dram_tensor("outp", [S, D], bf16, kind="ExternalOutput").ap()

    with tile.TileContext(nc) as tc:
        with (
            tc.tile_pool(name="const", bufs=1) as cpool,
            tc.tile_pool(name="kt", bufs=1) as kt_pool,
            tc.tile_pool(name="qt", bufs=1) as qt_pool,
            tc.tile_pool(name="vsb", bufs=1) as v_pool,
            tc.tile_pool(name="ctxn", bufs=1) as ctx_pool,
            tc.tile_pool(name="wkv", bufs=1) as wkv_pool,
            tc.tile_pool(name="xin", bufs=2) as xpool,
            tc.tile_pool(name="raw", bufs=8) as raw_pool,
            tc.tile_pool(name="t12", bufs=8) as t12_pool,
            tc.tile_pool(name="pp", bufs=8) as ppool,
            tc.tile_pool(name="sacc", bufs=6) as sacc_pool,
            tc.tile_pool(name="rcp", bufs=3) as rpool,
            tc.tile_pool(name="osb", bufs=6) as opool,
            # one PSUM pool for the whole kernel: 4 tags x 2 bufs = 8 banks
            tc.tile_pool(name="ps", space="PSUM", bufs=2) as pspool,
        ):
            def load_chunks(pool, name, nch, width, tag=None, eng=None,
                            groups=None):
                t = pool.tile([128, nch * width], bf16, tag=tag or name,
                              name=name + "_sb")
                dv = dram[name].rearrange("(n p) w -> p n w", p=128)
                if groups is None:
                    step = min(8, nch)
                    groups = [(i, step) for i in range(0, nch, step)]
                for i, step in groups:
                    e = eng or nc.sync
                    e.dma_start(t[:, i * width:(i + step) * width],
                                dv[:, i:i + step, :])
                return t

            def load(name, shape, dt=bf16, eng=None):
                t = cpool.tile(shape, dt, tag=name, name=name)
                (eng or nc.gpsimd).dma_start(t[:], dram[name][:])
                return t

            # Per-queue DMAs serialize (issue ~1.5us + transfer), so use
            # few, large transfers ordered by first consumption:
            #  scalar: wk (fine head), rmatT, trigT
            #  sync:   xkvT q0 chunks 0-7 (fine head), then x q1..q3
            #  gpsimd: xkvT q0 chunks 8-15, wv, then the WAR-gated wq and
            #          behind it wo/maskt/ones (transfer in the phase-1
            #          DMA lull, far ahead of their phase-2 consumers)
            # wk split across scalar + the otherwise-idle gpsimd queue so
            # the startup-critical weights land in half the time and wv
            # (behind wk on scalar) arrives before the first V projection
            wk_sb = load_chunks(wkv_pool, "wk", NDC, HW, eng=nc.scalar,
                                groups=[(0, 4), (4, 4)])
            dvk = dram["wk"].rearrange("(n p) w -> p n w", p=128)
            for i in (8, 12):
                nc.gpsimd.dma_start(wk_sb[:, i * HW:(i + 4) * HW],
                                    dvk[:, i:i + 4, :])
            rmatT = load("rmatT", [HD, HD], eng=nc.scalar)
            wv_sb = load_chunks(wkv_pool, "wv", NDC, HW, eng=nc.scalar,
                                groups=[(i, 4) for i in range(0, NDC, 4)])
            trigT = load("trigT", [HD, 2 * S], eng=nc.scalar)
            maskt = load("maskt", [128, 128], fp16, eng=nc.scalar)
            ones_sq = load("ones_sq", [128, 128], fp16, eng=nc.scalar)
            wo_sb = load_chunks(cpool, "wo", HW // 128, D, eng=nc.scalar,
                                groups=[(0, 2), (2, 2)])
            # wq reuses wk's buffer; its DMA WAR-waits on the last wk read
            # (end of the K projections) on the gpsimd queue, where the
            # stall blocks nothing (phase-2 accs ops start much later).
            wq_sb = load_chunks(wkv_pool, "wq", NDC, HW, tag="wk",
                                eng=nc.gpsimd,
                                groups=[(i, 4) for i in range(0, NDC, 4)])

            # per-head projection outputs (+rope for Q/K)
            kt_sb = [kt_pool.tile([128, S], bf16, tag=f"kt{h}", name=f"kt{h}")
                     for h in range(HPC)]
            qt_sb = [qt_pool.tile([128, S], bf16, tag=f"qt{h}", name=f"qt{h}")
                     for h in range(HPC)]
            v_sb = v_pool.tile([128, NKT * HW], fp16, tag="v", name="v_sb")
            ctx_sb = [ctx_pool.tile([128, S], bf16, tag=f"ctx{h}",
                                    name=f"ctx{h}") for h in range(HPC)]

            def rope(raw, out_sl, tq, rot_tag):
                ssl = slice(tq * 512, (tq + 1) * 512)
                csl = slice(S + tq * 512, S + (tq + 1) * 512)
                rot = pspool.tile([128, 512], f32, tag=rot_tag, name="rot")
                nc.tensor.matmul(rot[:], lhsT=rmatT[:], rhs=raw[:])
                t1 = t12_pool.tile([128, 512], bf16, tag="t1", name="t1")
                nc.vector.tensor_mul(t1[:], rot[:], trigT[:, ssl])
                t2 = t12_pool.tile([128, 512], bf16, tag="t2", name="t2")
                nc.vector.tensor_mul(t2[:], raw[:], trigT[:, csl])
                nc.vector.tensor_add(out_sl, t1[:], t2[:])

            def load_x(xname, tq):
                xt = xpool.tile([128, NDC * 512], bf16, tag="xin",
                                name=f"{xname}_{tq}")
                xv = dram[xname].rearrange("(n p) s -> p n s", p=128)
                for kc in range(0, NDC, 4):
                    nc.sync.dma_start(
                        xt[:, kc * 512:(kc + 4) * 512],
                        xv[:, kc:kc + 4, tq * 512:(tq + 1) * 512])
                return xt

            def mm_proj(ps, w_sb, xt, h, kc):
                nc.tensor.matmul(
                    ps[:],
                    lhsT=w_sb[:, kc * HW + h * HD:kc * HW + (h + 1) * HD],
                    rhs=xt[:, kc * 512:(kc + 1) * 512],
                    start=(kc == 0), stop=(kc == NDC - 1))

            def proj_units(xname, tq, xt, w_sb, out_tiles, acc_tag, rot_tag,
                           raw_eng=None):
                """Generator: 512-wide per-head projection (+rope) of token
                quarter tq, yielding after ~half-chain units.  The x DMA is
                issued by the caller (load_x) so the transfer overlaps the
                preceding block.  The PSUM->SBUF raw copy of chain h is
                emitted a full chain after h completes, so when these units
                are interleaved into the attention stream the copy's engine
                (Vector in phase 2, so Scalar stays exp-only) never waits on
                an unfinished chain and stalls unrelated attention ops
                queued behind it."""
                def finish(ps, h):
                    raw = raw_pool.tile([128, 512], bf16, tag="raw",
                                        name=f"raw_{xname}_{tq}_{h}")
                    if raw_eng == "vector":
                        nc.vector.tensor_copy(raw[:], ps[:])
                    else:
                        nc.scalar.copy(raw[:], ps[:])
                    rope(raw, out_tiles[h][:, tq * 512:(tq + 1) * 512],
                         tq, rot_tag)

                prev = None
                for h in range(HPC):
                    ps = pspool.tile([128, 512], f32, tag=acc_tag,
                                     name=f"ps_{xname}_{tq}_{h}")
                    for kc in range(8):
                        mm_proj(ps, w_sb, xt, h, kc)
                    yield
                    for kc in range(8, NDC):
                        mm_proj(ps, w_sb, xt, h, kc)
                    yield
                    if prev is not None:
                        finish(*prev)
                        yield
                    prev = (ps, h)
                finish(*prev)

            def proj_cold(tq, xt):
                """Chunk-major K projection + V for the startup quarter:
                consumes each chunk-group DMA as it lands.  K psums on
                A,A,B,B; rope rot on C; V chunk-major on C,C,D,D."""
                pss = [pspool.tile([128, 512], f32,
                                   tag=("A" if h < 2 else "B"),
                                   name=f"psc_{h}") for h in range(HPC)]
                for g in range(0, NDC, 4):
                    for h in range(HPC):
                        for kc in range(g, g + 4):
                            mm_proj(pss[h], wk_sb, xt, h, kc)
                for h in range(HPC):
                    raw = raw_pool.tile([128, 512], bf16, tag="raw",
                                        name=f"rawc_{h}")
                    nc.scalar.copy(raw[:], pss[h][:])
                    rope(raw, kt_sb[h][:, tq * 512:(tq + 1) * 512], tq, "C")
                # V chunk-major
                vps = [pspool.tile([128, 512], f32,
                                   tag=("C" if ti < 2 else "D"),
                                   name=f"vpsc_{ti}") for ti in range(4)]
                for g in range(0, NDC, 4):
                    for ti in range(4):
                        for kc in range(g, g + 4):
                            nc.tensor.matmul(
                                vps[ti][:],
                                lhsT=xt[:, kc * 512 + ti * 128:
                                        kc * 512 + (ti + 1) * 128],
                                rhs=wv_sb[:, kc * HW:(kc + 1) * HW],
                                start=(kc == 0), stop=(kc == NDC - 1))
                for ti in range(4):
                    t = tq * 4 + ti
                    nc.scalar.copy(v_sb[:, t * HW:(t + 1) * HW], vps[ti][:])

            def emit_v(tq, xt):
                for ti in range(4):
                    t = tq * 4 + ti
                    ps = pspool.tile([128, 512], f32, tag="C",
                                     name=f"vps_{t}")
                    for kc in range(NDC):
                        nc.tensor.matmul(
                            ps[:],
                            lhsT=xt[:, kc * 512 + ti * 128:
                                    kc * 512 + (ti + 1) * 128],
                            rhs=wv_sb[:, kc * HW:(kc + 1) * HW],
                            start=(kc == 0), stop=(kc == NDC - 1))
                    nc.scalar.copy(v_sb[:, t * HW:(t + 1) * HW], ps[:])

            def attention_block(qb, fill, ration=1):
                """Causal attention for query block qb; heads in pairs
                (ctx psums A,A / st C,C).  The PV matmuls lag one key-tile
                behind the logits and one dense fill unit is pulled between
                them, so the exp latency is covered twice over.  The softmax
                numerator accumulation runs on the idle Pool engine so the
                Vector stream stays short-latency (it gates PV via the
                causal mask) and Scalar stays exp-only."""
                qsl = slice(qb * 512, (qb + 1) * 512)
                last = 4 * qb + 3
                for hp in range(2):
                    pair = (2 * hp, 2 * hp + 1)
                    ctx_ps = {h: pspool.tile([128, 512], f32, tag="A",
                                             name=f"ctxps_{h}_{qb}")
                              for h in pair}
                    accs = {h: sacc_pool.tile([128, 512], fp16, tag="acc",
                                              name=f"acc_{h}_{qb}")
                            for h in pair}

                    def pv(kt, ps):
                        off = 128 * (kt - 4 * qb) if kt >= 4 * qb else 0
                        for h in pair:
                            nc.tensor.matmul(
                                ctx_ps[h][:, off:],
                                lhsT=v_sb[:, kt * HW + h * HD:
                                          kt * HW + (h + 1) * HD],
                                rhs=ps[h][:, off:], start=(kt == 0),
                                stop=(kt == last))

                    prev = None
                    for kt in range(last + 1):
                        off = 128 * (kt - 4 * qb) if kt >= 4 * qb else 0
                        cur = {}
                        for h in pair:
                            st = pspool.tile([128, 512], f32, tag="C",
                                             name=f"st_{h}_{qb}_{kt}")
                            nc.tensor.matmul(
                                st[:, off:],
                                lhsT=kt_sb[h][:, kt * 128:(kt + 1) * 128],
                                rhs=qt_sb[h][:, qb * 512 + off:
                                             (qb + 1) * 512])
                            p = ppool.tile([128, 512], fp16, tag="p",
                                           name=f"p_{h}_{qb}_{kt}")
                            nc.scalar.activation(p[:, off:], st[:, off:],
                                                 Exp, scale=SCALE)
                            if kt >= 4 * qb:
                                nc.vector.tensor_mul(
                                    p[:, off:off + 128],
                                    p[:, off:off + 128], maskt[:])
                            # numerator row-sum accumulation split across
                            # the Pool and Vector engines (Pool alone is
                            # ~2x slower per op and becomes the pacer)
                            acc_eng = nc.gpsimd if h % 2 == 0 else nc.vector
                            if kt == 0:
                                acc_eng.tensor_copy(accs[h][:], p[:])
                            else:
                                acc_eng.tensor_add(accs[h][:, off:],
                                                   accs[h][:, off:],
                                                   p[:, off:])
                            cur[h] = p
                        if (kt + hp) % ration == 0:
                            next(fill, None)
                        if prev is not None:
                            pv(*prev)
                        prev = (kt, cur)
                    pv(*prev)
                    next(fill, None)
                    for h in pair:
                        # one matmul: every partition of s_bc = row-sum of
                        # the softmax numerators (all-ones lhsT broadcasts)
                        s_bc = pspool.tile([128, 512], f32, tag="D",
                                           name=f"sbc_{h}_{qb}")
                        nc.tensor.matmul(s_bc[:], lhsT=ones_sq[:],
                                         rhs=accs[h][:])
                        rb_sb = rpool.tile([128, 512], f32, tag="rb",
                                           name=f"rbsb_{h}_{qb}")
                        nc.vector.reciprocal_approx_fast(rb_sb[:], s_bc[:])
                        nc.vector.tensor_mul(ctx_sb[h][:, qsl],
                                             ctx_ps[h][:], rb_sb[:])
                        next(fill, None)

            def wo_units(qb):
                for qt in range(qb * 4, qb * 4 + 4):
                    for db in range(NQB):
                        ps = pspool.tile([128, 512], f32, tag="D",
                                         name=f"ops_{qt}_{db}")
                        for h in range(HPC):
                            nc.tensor.matmul(
                                ps[:],
                                lhsT=ctx_sb[h][:, qt * 128:(qt + 1) * 128],
                                rhs=wo_sb[:, h * D + db * 512:
                                          h * D + (db + 1) * 512],
                                start=(h == 0), stop=(h == HPC - 1))
                        osb = opool.tile([128, 512], bf16, tag="o",
                                         name=f"osb_{qt}_{db}")
                        # scalar copy: by unit construction the wo psum is
                        # already complete, so this never blocks the exps
                        # queued behind it
                        nc.scalar.copy(osb[:], ps[:])
                        nc.sync.dma_start(
                            outp[qt * 128:(qt + 1) * 128,
                                 db * 512:(db + 1) * 512], osb[:])
                        yield

            def drain(g):
                for _ in g:
                    pass

            def chain(*gens):
                for g in gens:
                    yield from g

            # ---- phase 1: K^T + V (stream xkvT) ----
            proj_cold(0, load_x("xkvT", 0))
            for tq in range(1, NQB):
                xt = load_x("xkvT", tq)
                drain(proj_units("xkvT", tq, xt, wk_sb, kt_sb, "A", "D"))
                emit_v(tq, xt)

            # ---- phase 2: attention blocks q3..q0, with the next block's
            # Q-projection and the previous block's output projection
            # interleaved into each block's exp-paced stream ----
            xq3 = load_x("xqT", 3)
            drain(proj_units("xqT", 3, xq3, wq_sb, qt_sb, "B", "D"))
            xq2 = load_x("xqT", 2)
            fill = proj_units("xqT", 2, xq2, wq_sb, qt_sb, "B", "D",
                              raw_eng="vector")
            # only 12 fill units for 36 pull sites: ration so they stretch
            # to the block-end row-sum waits
            attention_block(3, fill, ration=3)
            drain(fill)
            xq1 = load_x("xqT", 1)
            fill = chain(wo_units(3),
                         proj_units("xqT", 1, xq1, wq_sb, qt_sb, "B", "D",
                                    raw_eng="vector"))
            attention_block(2, fill)
            drain(fill)
            xq0 = load_x("xqT", 0)
            fill = chain(wo_units(2),
                         proj_units("xqT", 0, xq0, wq_sb, qt_sb, "B", "D",
                                    raw_eng="vector"))
            attention_block(1, fill)
            drain(fill)
            fill = wo_units(1)
            attention_block(0, fill)
            drain(fill)
            drain(wo_units(0))

    nc.compile()
    return nc


def _host_constants():
    # sin/cos tables exactly as the flaxformer reference (fp32 math)
    fraction = np.arange(0, HD, 2, dtype=np.float32) / np.float32(HD)
    timescale = (np.float32(10000.0) ** fraction).astype(np.float32)
    sinusoid = np.einsum(
        "i,j->ij", np.arange(S, dtype=np.float32),
        (np.float32(1.0) / timescale)).astype(np.float32)
    sinusoid = np.concatenate([sinusoid, sinusoid], axis=-1)  # [S, HD]
    trigT = np.concatenate([np.sin(sinusoid), np.cos(sinusoid)],
                           axis=0).astype(np.float32).T.copy()  # [HD, 2S]

    # rotate_half as a matmul: rot = R @ x, lhsT = R^T
    R = np.zeros((HD, HD), np.float32)
    for i in range(64):
        R[i, i + 64] = -1.0
        R[i + 64, i] = 1.0

    # causal triangle for the 128-wide diagonal sub-block: allowed iff c >= r
    r = np.arange(128)[:, None]
    c = np.arange(128)[None, :]
    maskt = (c - r >= 0).astype(np.float32)

    return {
        "trigT": trigT.astype(BF16),
        "rmatT": R.T.copy().astype(BF16),
        "ones_sq": np.ones((128, 128), np.float16),
        "maskt": maskt.astype(np.float16),
    }


def kernel(inputs_q, inputs_kv, wq, wk, wv, wo, mask=None):
    _install_ntff_hook()
    from concourse import bass_utils

    if "nc" not in _CACHE:
        _CACHE["nc"] = _build()
        _CACHE["consts"] = _host_constants()
    nc = _CACHE["nc"]
    consts = _CACHE["consts"]

    wq2 = np.asarray(wq, np.float32).reshape(D, H * HD)
    wk2 = np.asarray(wk, np.float32).reshape(D, H * HD)
    wv2 = np.asarray(wv, np.float32).reshape(D, H * HD)
    wo2 = np.asarray(wo, np.float32).reshape(H * HD, D)
    xq = np.asarray(inputs_q, np.float32)
    xkv = np.asarray(inputs_kv, np.float32)

    in_maps = []
    for c in range(N_CORES):
        b, hg = divmod(c, H // HPC)
        hs = slice(hg * HW, (hg + 1) * HW)
        in_maps.append({
            "xqT": np.ascontiguousarray(xq[b].T).astype(BF16),
            "xkvT": np.ascontiguousarray(xkv[b].T).astype(BF16),
            "wq": wq2[:, hs].astype(BF16),
            "wk": wk2[:, hs].astype(BF16),
            "wv": wv2[:, hs].astype(BF16),
            "wo": wo2[hs, :].astype(BF16),
            **consts,
        })

    trace = bool(int(os.environ.get("KERNEL_TRACE", "0")))
    res = bass_utils.run_bass_kernel_spmd(
        nc, in_maps, core_ids=list(range(N_CORES)), trace=trace)
    _CACHE["last_result"] = res

    out = np.zeros((B, S, D), np.float32)
    for c in range(N_CORES):
        out[c // (H // HPC)] += np.asarray(res.results[c]["outp"], np.float32)
    return out
